# revision 25
# baseline (speedup 1.0000x reference)
"""Trainium2 Bass kernel for nn_ConditionalJiTBlock (DiT-style block with
AdaLN modulation, self-attention, cross-attention and SwiGLU FFN).

Sharding: 8 NeuronCores = 4 batch elements x 2 token-halves. Each core
computes its 512 query tokens end-to-end with zero collectives; the K/V
projections (which need all 1024 tokens of the batch element) are
replicated within each pair of cores. SPMD safety: the host permutes each
core's token axis so the core's local tokens are always columns 0..511 of
the on-chip tensors (attention is permutation-invariant over key tokens).

Layout: activations are feature-major on chip (features on partitions,
tokens on the free axis). Projection dtypes are mixed for speed at bounded
accuracy cost (validated against an fp64-path numpy sim):
  - qkv / cross-q / cross-kv weights + their activations: fp8e4m3 with
    DoubleRow matmuls (two contraction k-tiles per instruction).
  - ada / o-proj / cross-o / FFN weights: fp8e3m4 scaled x64 on host
    (weights are ~N(0, 0.02); the scale moves them out of e3m4's subnormal
    range), activations bf16/fp8; the 1/64 is folded into the PSUM
    evacuation affine. Attention q/k/scores/softmax/v stays bf16/f32.
Softmax: the two heads of a q/k tile are emitted back-to-back so their
score matmuls run concurrently in different PE row groups (base partition
0 / 64); exp is batched over 2-bank PSUM tiles; P@V chases the exp stream
one head-pair behind. The denominator comes from the interleaved
ones-column of V (PSUM row 64) and is broadcast across partitions with a
K=2 selector matmul.
"""

import numpy as np
import ml_dtypes

BF16 = ml_dtypes.bfloat16
E4 = ml_dtypes.float8_e4m3
E3 = ml_dtypes.float8_e3m4

B, N, M, D, H, HD = 4, 1024, 1024, 1024, 16, 64
MH = 2730
MHP = 2816          # MH padded to 22*128
EPS = 1e-6
NCORES = 8
T = 512             # local query tokens per core
DT = D // 128       # 8
KP = DT // 2        # 4 contraction k-pairs for DoubleRow
FHT = MHP // 128    # 22
NMOD = 9
ATT_SCALE = HD ** -0.5
WS = 64.0           # host scale applied to e3m4 weights


# ==========================================================================
# device graph
# ==========================================================================

def build_graph(sim_compat=False):
    import concourse.bacc as bacc
    import concourse.mybir as mybir
    import concourse.tile as tile

    F32 = mybir.dt.float32
    BT = mybir.dt.bfloat16
    F8 = mybir.dt.float8e4
    F8E3 = mybir.dt.float8e3

    nc = bacc.Bacc("TRN2", target_bir_lowering=False, debug=False,
                   num_devices=NCORES)

    def din(name, shape, dtype):
        return nc.dram_tensor(name, shape, dtype, kind="ExternalInput").ap()

    p = {}
    # activations
    p["xt"] = din("xt", [D, N], BT)          # x[b].T, local tokens first
    p["xres"] = din("xres", [D, T], F32)     # f32 residual columns (local)
    p["srct"] = din("srct", [512, 2 * M], F8)  # src^T, k-pair interleaved
    p["cvec"] = din("cvec", [D, 1], F32)     # c[b]
    # weights
    p["ada"] = din("ada", [D, NMOD * D], F8E3)      # x64
    p["wqkv"] = din("wqkv", [512, 2 * 3 * D], F8)   # k-pair interleaved
    p["wo"] = din("wo", [512, 2 * D], F8)           # k-pair interleaved
    p["wcq"] = din("wcq", [512, 2 * D], F8)
    p["wckv"] = din("wckv", [512, 2 * 2 * D], F8)
    p["wco"] = din("wco", [512, 2 * D], F8)         # k-pair interleaved
    p["w1"] = din("w1", [512, 2 * MHP], F8)         # k-pair interleaved
    p["w2"] = din("w2", [512, 2 * MHP], F8)         # k-pair interleaved
    p["w3"] = din("w3", [MHP, D], F8E3)             # x64
    # feature-major f32 vectors [128, k]  (column j = feature tile j)
    p["adab"] = din("adab", [128, NMOD * DT], F32)
    p["n1w"] = din("n1w", [128, DT], F32)
    p["ncw"] = din("ncw", [128, DT], F32)
    p["n2w"] = din("n2w", [128, DT], F32)
    p["qkvb"] = din("qkvb", [128, 3 * DT], F32)
    p["obf"] = din("obf", [128, DT], F32)    # sa_o_b + v_bias @ Wo (host fold)
    p["cqb"] = din("cqb", [128, DT], F32)
    p["ckb"] = din("ckb", [128, DT], F32)    # cross-k bias
    p["cobf"] = din("cobf", [128, DT], F32)  # ca_o_b + cross-v bias @ Wco
    p["b1f"] = din("b1f", [128, FHT], F32)
    p["b2f"] = din("b2f", [128, FHT], F32)
    p["b3f"] = din("b3f", [128, DT], F32)
    # constant selector matrices, bf16
    p["ones128"] = din("ones128", [128, 128], BT)
    p["bd16"] = din("bd16", [128, 128], BT)
    p["sel2"] = din("sel2", [1, 256], BT)
    p["qsel"] = din("qsel", [16, D], BT)
    p["ksel"] = din("ksel", [16, D], BT)
    p["cqsel"] = din("cqsel", [16, D], BT)
    p["cksel"] = din("cksel", [16, D], BT)

    p["out"] = nc.dram_tensor("out", [D, T], F32, kind="ExternalOutput").ap()

    with tile.TileContext(nc) as tc:
        _emit(nc, tc, p, mybir, sim_compat)
    nc.compile()
    return nc


def _emit(nc, tc, p, mybir, sim_compat=False):
    ALU = mybir.AluOpType
    ACTF = mybir.ActivationFunctionType
    DR = mybir.MatmulPerfMode.DoubleRow
    F32 = mybir.dt.float32
    BT = mybir.dt.bfloat16
    F8 = mybir.dt.float8e4
    F8E3 = mybir.dt.float8e3
    F8E5 = mybir.dt.float8e5

    pg = tc.alloc_tile_pool(name="pg", bufs=1)
    ps = tc.alloc_tile_pool(name="ps", bufs=1, space="PSUM")
    dram = tc.alloc_tile_pool(name="dram", bufs=1, space="DRAM")

    # shared-tag allocators; slots rotate by liveness within each tag
    def bigw(name):   # wide bf16 activations (xt / k / v tiles)
        return pg.tile([128, 1040], BT, tag="bigw", name=name, bufs=24)

    def xf(name):     # f32 [128, T] residual-stream tiles
        return pg.tile([128, T], F32, tag="xf", name=name, bufs=16)

    def qt(name):     # bf16 [128, T] q tiles
        return pg.tile([128, T], BT, tag="qt", name=name, bufs=8)

    def op8(name):    # attention-out pair tiles [128, 2*512] fp8e4
        return pg.tile([128, 2 * 512], F8, tag="op8", name=name, bufs=4)

    def f8p(name):    # fp8 k-pair activation tiles [128, 2*1024]
        return pg.tile([128, 2 * 1024], F8, tag="f8p", name=name, bufs=8)

    def wgt8(name):   # fp8e4 DoubleRow weight tiles [128, 2*1024]
        return pg.tile([128, 2 * 1024], F8, tag="wgt8", name=name, bufs=8)

    def wgt3(name, wid=1024):  # fp8e3 weight stream tiles
        return pg.tile([128, wid], F8E3, tag="wgt3", name=name, bufs=18)

    def pt2(name):    # exp(p) tiles [128, 1024] fp8e5 (kt-pair layout)
        return pg.tile([128, 1024], F8E5, tag="pt", name=name, bufs=6)

    def h2b(name):    # FFN h2 tiles [128, 1024] bf16
        return pg.tile([128, 1024], BT, tag="h2b", name=name, bufs=1)

    def vp8(name):    # paired V tiles [128, 2*1280] fp8e4 (kt-pair layout,
        # heads at stride 80 so DoubleRow weight offsets stay 16-aligned)
        return pg.tile([128, 2 * 1280], F8, tag="vp8", name=name, bufs=8)

    def hsb(name):    # FFN h tiles, fp8e4
        return pg.tile([128, T], F8, tag="h_sb", name=name, bufs=FHT)

    def sqt(name):    # bf16 [128, 512] scratch (squares, norm tmp, h1 tmp)
        return pg.tile([128, 512], BT, tag="sq", name=name, bufs=2)

    def scratch4k(name, rows=128, wid=1024):  # f32 scratch (rr/ssq)
        return pg.tile([rows, wid], F32, tag="s4k", name=name, bufs=1)

    def scrbf(name, rows=16, wid=1024):
        return pg.tile([rows, wid], BT, tag="sbf", name=name, bufs=1)

    def rbt(name):    # den broadcast tiles [128, T] bf16
        return pg.tile([128, T], BT, tag="rbt", name=name, bufs=1)

    def rdt(name):    # den strips [1, 2T] f32 (head A | head B)
        return pg.tile([1, 2 * T], F32, tag="rdt", name=name, bufs=1)

    def rdbt(name):   # den strips [1, 2T] bf16
        return pg.tile([1, 2 * T], BT, tag="rdbt", name=name, bufs=1)

    def selt(name):   # on-demand qk-norm selector tiles [16, D]
        return pg.tile([16, D], BT, tag="selt", name=name, bufs=2)

    def psA(name):    # 2-bank psum [128, 1024]
        return ps.tile([128, 1024], F32, tag="psA", name=name, bufs=2)

    def psB(name):    # 1-bank psum [128, 512]
        return ps.tile([128, 512], F32, tag="psB", name=name, bufs=4)

    def pair(t):
        """view [128, 2*W] tile as [128, 2, W]"""
        return t[:].rearrange("p (j f) -> p j f", j=2)

    # ---------------- constants ----------------
    cst = {}
    c_eps = pg.tile([128, 1], F32, tag="c_eps", name="c_eps")
    nc.any.memset(c_eps[:], EPS)
    for nm, np_, k in (("ones128", 128, 128), ("bd16", 128, 128),
                       ("sel2", 1, 256)):
        t = pg.tile([np_, k], BT, tag=nm, name=f"c_{nm}")
        nc.sync.dma_start(t[:], p[nm][:])
        cst[nm] = t
    for nm, k in (("adab", NMOD * DT), ("n1w", DT), ("ncw", DT), ("n2w", DT),
                  ("qkvb", 3 * DT), ("obf", DT), ("cqb", DT), ("ckb", DT),
                  ("cobf", DT), ("b1f", FHT), ("b2f", FHT), ("b3f", DT)):
        t = pg.tile([128, k], F32, tag=nm, name=f"c_{nm}")
        nc.sync.dma_start(t[:], p[nm][:])
        cst[nm] = t

    def load_sel(nm):
        t = selt(f"c_{nm}")
        nc.scalar.dma_start(t[:], p[nm][:])
        return t

    # =====================================================================
    # Stage 0: x/c loads first, then the ada stream (fp8e3).  RMS(x) stats
    # (mods-independent) are interleaved with the ada matmuls so the PE
    # stays busy while ada weight groups stream in.
    # =====================================================================
    xt_sb = []
    for k in range(DT):
        t = bigw(f"xt{k}")
        nc.sync.dma_start(t[:, 0:N], p["xt"][k * 128:(k + 1) * 128, :])
        xt_sb.append(t)
    cv = pg.tile([128, DT], F32, tag="cv", name="cv")
    nc.sync.dma_start(cv[:], p["cvec"][:].rearrange("(k p) o -> p (k o)", p=128))
    sc = pg.tile([128, DT], BT, tag="sc", name="sc")
    nc.scalar.activation(sc[:], cv[:], ACTF.Sigmoid)
    nc.vector.tensor_tensor(sc[:], sc[:], cv[:], ALU.mult)

    # n1 RMS sum-of-squares, interleaved unit-by-unit with the ada groups
    n1_pss = [psA("ssn_n1a"), psA("ssn_n1b")]

    def n1_units():
        for k in range(DT):
            for c in range(2):
                sq = sqt(f"sq_n1{k}_{c}")
                nc.vector.tensor_tensor(sq[:], xt_sb[k][:, c * 512:(c + 1) * 512],
                                        xt_sb[k][:, c * 512:(c + 1) * 512],
                                        ALU.mult)
                nc.tensor.matmul(n1_pss[c][:, 0:512], cst["ones128"][:], sq[:],
                                 start=(k == 0), stop=(k == DT - 1))
            yield

    n1_gen = n1_units()

    NCH_ADA = NMOD * D // 512  # 18
    scr = dram.tile([NMOD * D], F32, tag="scr", name="scr")
    scr_ch = scr[:].rearrange("(a b) -> a b", a=NCH_ADA)
    for grp in range(NMOD):  # 9 groups of 1024 ada columns
        gtiles = []
        eng = nc.sync if grp % 2 == 0 else nc.scalar
        for k in range(DT):
            at = wgt3(f"ada_g{grp}_{k}")
            eng.dma_start(
                at[:], p["ada"][k * 128:(k + 1) * 128, grp * D:(grp + 1) * D])
            gtiles.append(at)
        for ch in range(2):
            gi = grp * 2 + ch
            pm = psB(f"pm{gi}")
            for k in range(DT):
                nc.tensor.matmul(pm[0:1, :], sc[:, k:k + 1],
                                 gtiles[k][:, ch * 512:(ch + 1) * 512],
                                 start=(k == 0), stop=(k == DT - 1))
            strip = pg.tile([1, 512], F32, tag="xf", name=f"mstrip{gi}",
                            bufs=16)
            nc.scalar.activation(strip[:], pm[0:1, :], ACTF.Identity,
                                 scale=1.0 / WS)
            nc.scalar.dma_start(scr_ch[gi:gi + 1, :], strip[:])
        next(n1_gen, None)

    for _ in n1_gen:
        pass

    seff = {}
    gb = {}
    g64 = {}
    _msl = []

    def emit_mods():
        """Reload mods feature-major, derive the per-sublayer vectors.
        Called after the n1 product pass so that pass overlaps the
        DRAM bounce."""
        mods = pg.tile([128, NMOD * DT], F32, tag="mods", name="mods")
        nc.scalar.dma_start(mods[:], scr[:].rearrange("(j p) -> p j", p=128))
        nc.vector.tensor_tensor(mods[:], mods[:], cst["adab"][:], ALU.add)

        def msl(i):  # mods columns of modulation param i
            return mods[:, i * DT:(i + 1) * DT]

        _msl.append(msl)
        for nm, i_scale, w in (("sa", 1, "n1w"), ("ca", 4, "ncw"),
                               ("ff", 7, "n2w")):
            s1 = pg.tile([128, DT], F32, tag=f"seff_{nm}", name=f"seff_{nm}")
            nc.vector.tensor_scalar(s1[:], msl(i_scale), 1.0, None, ALU.add)
            nc.vector.tensor_tensor(s1[:], s1[:], cst[w][:], ALU.mult)
            seff[nm] = s1
        for nm, i_gate, bias in (("sa", 2, "obf"), ("ca", 5, "cobf"),
                                 ("ff", 8, "b3f")):
            t = pg.tile([128, DT], F32, tag=f"gb_{nm}", name=f"gb_{nm}")
            nc.vector.tensor_tensor(t[:], msl(i_gate), cst[bias][:], ALU.mult)
            gb[nm] = t
        # gate columns pre-divided by WS (the e3m4 weight scale) for the
        # o-proj / cross-o / FFN-out PSUM evacuations
        for nm, i_gate in (("sa", 2), ("ca", 5), ("ff", 8)):
            t = pg.tile([128, DT], F32, tag=f"g64_{nm}", name=f"g64_{nm}")
            nc.vector.tensor_scalar(t[:], msl(i_gate), 1.0 / WS, None, ALU.mult)
            g64[nm] = t
        return msl

    sh_col = {"sa": 0, "ca": 3, "ff": 6}

    # =====================================================================
    # helpers
    # =====================================================================
    def load_wgroup_dr(w_name, cols0, cols, tagname):
        """fp8e4 DoubleRow weight tiles: KP tiles [128, 2, cols]."""
        tiles = []
        for kp in range(KP):
            t = wgt8(f"{tagname}_{kp}")
            nc.sync.dma_start(
                pair(t)[:, :, 0:cols],
                p[w_name][kp * 128:(kp + 1) * 128, :]
                .rearrange("p (j f) -> p j f", j=2)[:, :, cols0:cols0 + cols])
            tiles.append(t)
        return tiles

    def load_wgroup3(w_name, cols0, cols, tagname):
        """fp8e3 plain weight tiles, one per contraction k-tile."""
        tiles = []
        for k in range(DT):
            t = wgt3(f"{tagname}_{k}", wid=cols)
            nc.sync.dma_start(
                t[:, 0:cols], p[w_name][k * 128:(k + 1) * 128, cols0:cols0 + cols])
            tiles.append(t)
        return tiles

    def norm_mod(xtiles, Ttok, seff_t, sh_slice, name, writer, pss=None,
                 defer_mod=False):
        """RMS + AdaLN modulate of feature-major tiles.

        writer(k) -> destination AP [128, Ttok] for the k-tile.  The
        x*(1/rms) product is written first; the AdaLN affine is applied
        in place.  With defer_mod=True the affine pass is returned as a
        closure so the product pass can overlap work that does not yet
        have the modulation vectors.
        """
        NCH = Ttok // 512
        if pss is None:
            pp = psA(f"ssn_{name}")
            for k in range(DT):
                for c in range(NCH):
                    sq = sqt(f"sq_{name}{k}_{c}")
                    nc.vector.tensor_tensor(sq[:], xtiles[k][:, c * 512:(c + 1) * 512],
                                            xtiles[k][:, c * 512:(c + 1) * 512],
                                            ALU.mult)
                    nc.tensor.matmul(pp[:, c * 512:(c + 1) * 512],
                                     cst["ones128"][:], sq[:],
                                     start=(k == 0), stop=(k == DT - 1))
            pss = [pp[:, c * 512:(c + 1) * 512] for c in range(NCH)]
        rr = scratch4k(f"rr_{name}")
        for c in range(NCH):
            nc.scalar.activation(rr[:, c * 512:(c + 1) * 512], pss[c],
                                 ACTF.Sqrt, bias=c_eps[:], scale=1.0 / D)
        nc.vector.reciprocal_approx_fast(rr[:, 0:Ttok], rr[:, 0:Ttok])
        outs = []
        for k in range(DT):
            dst = writer(k)
            nc.vector.tensor_tensor(dst[:, 0:Ttok], xtiles[k][:, 0:Ttok],
                                    rr[:, 0:Ttok], ALU.mult)
            outs.append(dst)

        def modulate(seff_t, sh_slice):
            for k in range(DT):
                if k % 2 == 0:
                    nc.vector.tensor_scalar(outs[k][:, 0:Ttok], outs[k][:, 0:Ttok],
                                            seff_t[:, k:k + 1], sh_slice[:, k:k + 1],
                                            ALU.mult, ALU.add)
                else:
                    nc.scalar.activation(outs[k][:, 0:Ttok], outs[k][:, 0:Ttok],
                                         ACTF.Identity, bias=sh_slice[:, k:k + 1],
                                         scale=seff_t[:, k:k + 1])
            return outs

        if defer_mod:
            return modulate
        return modulate(seff_t, sh_slice)

    def qk_norm(qtiles, Ttok, sel_t, name, filler=None):
        """Per-head RMS norm in place; head-norm weight folded into sel."""
        NCH = Ttok // 512
        ssq = scratch4k(f"ssq_{name}", rows=16)
        for c in range(NCH):
            pq = psB(f"psq_{name}{c}")
            for t in range(DT):
                sq = sqt(f"qs_{name}{t}_{c}")
                nc.vector.tensor_tensor(sq[:], qtiles[t][:, c * 512:(c + 1) * 512],
                                        qtiles[t][:, c * 512:(c + 1) * 512],
                                        ALU.mult)
                nc.tensor.matmul(pq[0:16, :],
                                 cst["bd16"][:, t * 16:(t + 1) * 16], sq[:],
                                 start=(t == 0), stop=(t == DT - 1))
            nc.scalar.activation(ssq[:, c * 512:(c + 1) * 512], pq[0:16, :],
                                 ACTF.Sqrt, bias=c_eps[0:16, :], scale=1.0 / HD)
        nc.vector.reciprocal_approx_fast(ssq[:, 0:Ttok], ssq[:, 0:Ttok])
        rqb = scrbf(f"rqb_{name}")
        nc.vector.tensor_copy(rqb[:, 0:Ttok], ssq[:, 0:Ttok])
        for t in range(DT):
            for c in range(NCH):
                pb = psB(f"qb_{name}{t}_{c}")
                nc.tensor.matmul(pb[:], sel_t[:, t * 128:(t + 1) * 128],
                                 rqb[:, c * 512:(c + 1) * 512],
                                 start=True, stop=True)
                nc.vector.tensor_tensor(qtiles[t][:, c * 512:(c + 1) * 512],
                                        qtiles[t][:, c * 512:(c + 1) * 512],
                                        pb[:], ALU.mult)
            if filler is not None:
                filler()

    def attention(q_sb, k_sb, v_sb, Tk, name, filler=None):
        """softmax(q k^T / 8) v with a (t, kp)-unit software pipeline."""
        KTk = Tk // 128
        NKP = KTk // 2
        o_pair = [op8(f"o_{name}{m}") for m in range(DT // 2)]

        def o_half(t):
            return o_pair[t // 2][:, (t % 2) * 512:(t % 2) * 512 + 512]
        pts = {}
        po_cur = {}
        rd_cur = {}

        def emit_S(t, kp):
            spA = psA(f"sA_{name}{t}_{kp}")
            spB = psA(f"sB_{name}{t}_{kp}")
            for j in (0, 1):
                kt = 2 * kp + j
                nc.tensor.matmul(spA[:, j * 512:(j + 1) * 512],
                                 k_sb[t][0:64, kt * 128:(kt + 1) * 128],
                                 q_sb[t][0:64, 0:T], start=True, stop=True)
                nc.tensor.matmul(spB[:, j * 512:(j + 1) * 512],
                                 k_sb[t][64:128, kt * 128:(kt + 1) * 128],
                                 q_sb[t][64:128, 0:T], start=True, stop=True)
            pA = pt2(f"ptA_{name}{t}_{kp}")
            pB = pt2(f"ptB_{name}{t}_{kp}")
            nc.scalar.activation(pA[:], spA[:], ACTF.Exp, scale=ATT_SCALE)
            nc.scalar.activation(pB[:], spB[:], ACTF.Exp, scale=ATT_SCALE)
            pts[(t, kp)] = (pA, pB)

        def emit_PV(t, kp):
            if kp == 0:
                po_cur[t] = (psB(f"poA_{name}{t}"), psB(f"poB_{name}{t}"))
            poA, poB = po_cur[t]
            pA, pB = pts.pop((t, kp))
            for po, pt_, hp in ((poA, pA, 0), (poB, pB, 1)):
                h16 = 2 * t + hp
                nc.tensor.matmul(po[0:65, :],
                                 pair(v_sb[kp])[:, :, h16 * 80:h16 * 80 + 65],
                                 pt_[:].rearrange("p (j f) -> p j f", j=2),
                                 start=(kp == 0), stop=(kp == NKP - 1),
                                 perf_mode=DR)

        def emit_EPI(t):
            # NOTE: custom DVE ops (reciprocal_approx_fast) ignore the
            # operand base partition on HW — copy PSUM row 64 to a
            # partition-0 strip first, then recip in place.
            poA, poB = po_cur.pop(t)
            rd = rdt(f"rd_{name}{t}")
            nc.vector.tensor_copy(rd[:, 0:T], poA[64:65, 0:T])
            nc.vector.tensor_copy(rd[:, T:2 * T], poB[64:65, 0:T])
            nc.vector.reciprocal_approx_fast(rd[:], rd[:])
            rdb = rdbt(f"rdb_{name}{t}")
            nc.vector.tensor_copy(rdb[:], rd[:])
            bc = psB(f"bc_{name}{t}")
            nc.tensor.matmul(bc[:], cst["sel2"][:, 0:128], rdb[:, 0:T],
                             start=True, stop=False)
            nc.tensor.matmul(bc[:], cst["sel2"][:, 128:256], rdb[:, T:2 * T],
                             start=False, stop=True)
            rb = rbt(f"rb_{name}{t}")
            nc.scalar.activation(rb[:], bc[:], ACTF.Identity)
            oh = o_half(t)
            nc.vector.tensor_tensor(oh[0:64, :], poA[0:64, :],
                                    rb[0:64, :], ALU.mult)
            nc.vector.tensor_tensor(oh[64:128, :], poB[0:64, :],
                                    rb[64:128, :], ALU.mult)

        units = [(t, kp) for t in range(DT) for kp in range(NKP)]
        for i in range(len(units) + 2):
            if i < len(units):
                emit_S(*units[i])
            if i >= 2:
                t, kp = units[i - 2]
                emit_PV(t, kp)
                if kp == NKP - 1:
                    emit_EPI(t)
                if filler is not None:
                    filler()
        return o_pair

    def proj_dr(w_name, wcols0, xn_p, Tt, bias, bias0, name, alloc, n_f=DT):
        """Feature-major fp8 DoubleRow projection over n_f output tiles."""
        outs = []
        NCH = Tt // 512
        for f0 in range(0, n_f, 8):
            nf = min(8, n_f - f0)
            wt = load_wgroup_dr(w_name, wcols0 + f0 * 128, nf * 128,
                                f"{name}_w{f0}")
            for f in range(nf):
                o = alloc(f"{name}_o{f0 + f}")
                outs.append(o)
                for c in range(NCH):
                    pp = psB(f"p_{name}{f0 + f}_{c}")
                    for kp in range(KP):
                        nc.tensor.matmul(
                            pp[:],
                            pair(wt[kp])[:, :, f * 128:(f + 1) * 128],
                            pair(xn_p[kp])[:, :, c * 512:(c + 1) * 512],
                            start=(kp == 0), stop=(kp == KP - 1),
                            perf_mode=DR)
                    nc.scalar.activation(
                        o[:, c * 512:(c + 1) * 512], pp[:], ACTF.Identity,
                        bias=bias[:, bias0 + f0 + f:bias0 + f0 + f + 1])
        return outs

    def proj_tok_dr(w_name, wcols0, xn_p, Tt, name):
        """Token-major fp8 DoubleRow V projection into kt-paired tiles
        with an interleaved ones column per head (position 64 of each
        80-wide head block)."""
        ntt = Tt // 128
        wt = load_wgroup_dr(w_name, wcols0, D, f"{name}_w")
        outs = []
        for m in range(ntt // 2):
            o = vp8(f"{name}_v{m}")
            nc.any.memset(o[:], 1.0)
            outs.append(o)
        for tt in range(ntt):
            for c in range(2):
                pp = psB(f"pv_{name}{tt}_{c}")
                for kp in range(KP):
                    nc.tensor.matmul(
                        pp[:],
                        pair(xn_p[kp])[:, :, tt * 128:(tt + 1) * 128],
                        pair(wt[kp])[:, :, c * 512:(c + 1) * 512],
                        start=(kp == 0), stop=(kp == KP - 1),
                        perf_mode=DR)
                half = outs[tt // 2][:, (tt % 2) * 1280:(tt % 2) * 1280 + 1280]
                dst = half[:, c * 8 * 80:(c * 8 + 8) * 80].rearrange(
                    "p (g e) -> p g e", g=8)[:, :, 0:64]
                src = pp[:].rearrange("p (g e) -> p g e", g=8)
                nc.vector.tensor_copy(dst, src)
        return outs

    # =====================================================================
    # Stage 1: self-attention sublayer
    # =====================================================================
    # normalized x in fp8 k-pair layout for the DoubleRow projections;
    # the x*(1/rms) pass runs while the mods DRAM bounce is in flight
    xn_p = [f8p(f"xnp{kp}") for kp in range(KP)]
    n1_modulate = norm_mod(xt_sb, N, None, None, "n1",
                           lambda k: pair(xn_p[k // 2])[:, k % 2, :],
                           pss=[n1_pss[c][:, 0:512] for c in range(2)],
                           defer_mod=True)
    msl = emit_mods()
    n1_modulate(seff["sa"], msl(sh_col["sa"]))

    q_sa = proj_dr("wqkv", 0, xn_p, T, cst["qkvb"], 0, "qsa", qt)

    xres_sb = []
    for k in range(DT):
        t = xf(f"xres{k}")
        nc.scalar.dma_start(t[:], p["xres"][k * 128:(k + 1) * 128, :])
        xres_sb.append(t)
    k_sa = proj_dr("wqkv", D, xn_p, N, cst["qkvb"], DT, "ksa", bigw)
    v_sa = proj_tok_dr("wqkv", 2 * D, xn_p, N, "vsa")

    # source tokens + cross-KV as filler units (qk-norm gaps + attention-1)
    srct_p = []
    for kp in range(KP):
        t = f8p(f"srctp{kp}")
        nc.scalar.dma_start(t[:, 0:2 * M], p["srct"][kp * 128:(kp + 1) * 128, :])
        srct_p.append(t)

    kca = [bigw(f"kca_o{f}") for f in range(DT)]
    vca = []
    for m in range(M // 256):
        o = vp8(f"vca_v{m}")
        nc.any.memset(o[:], 1.0)
        vca.append(o)

    def ckv_units():
        wt = load_wgroup_dr("wckv", 0, D, "kca_w")
        for f in range(DT):
            for c in range(2):
                pp = psB(f"p_kca{f}_{c}")
                for kp in range(KP):
                    nc.tensor.matmul(
                        pp[:], pair(wt[kp])[:, :, f * 128:(f + 1) * 128],
                        pair(srct_p[kp])[:, :, c * 512:(c + 1) * 512],
                        start=(kp == 0), stop=(kp == KP - 1), perf_mode=DR)
                nc.scalar.activation(kca[f][:, c * 512:(c + 1) * 512], pp[:],
                                     ACTF.Identity, bias=cst["ckb"][:, f:f + 1])
                yield
        wtv = load_wgroup_dr("wckv", D, D, "vca_w")
        for tt in range(M // 128):
            for c in range(2):
                pp = psB(f"pv_vca{tt}_{c}")
                for kp in range(KP):
                    nc.tensor.matmul(
                        pp[:], pair(srct_p[kp])[:, :, tt * 128:(tt + 1) * 128],
                        pair(wtv[kp])[:, :, c * 512:(c + 1) * 512],
                        start=(kp == 0), stop=(kp == KP - 1), perf_mode=DR)
                half = vca[tt // 2][:, (tt % 2) * 1280:(tt % 2) * 1280 + 1280]
                dst = half[:, c * 8 * 80:(c * 8 + 8) * 80].rearrange(
                    "p (g e) -> p g e", g=8)[:, :, 0:64]
                nc.vector.tensor_copy(dst, pp[:].rearrange("p (g e) -> p g e", g=8))
                yield

    ckv_gen = ckv_units()

    def ckv_filler():
        next(ckv_gen, None)

    qk_norm(q_sa, T, load_sel("qsel"), "qsa", filler=ckv_filler)
    qk_norm(k_sa, N, load_sel("ksel"), "ksa", filler=ckv_filler)

    wo_t = load_wgroup_dr("wo", 0, D, "wo")
    o1 = attention(q_sa, k_sa, v_sa, N, "a1", filler=ckv_filler)
    for _ in ckv_gen:
        pass

    x1 = []
    nc_pss = psA("ssn_nc")
    for f in range(DT):
        pp = psB(f"po1_{f}")
        for kp in range(KP):
            nc.tensor.matmul(pp[:], pair(wo_t[kp])[:, :, f * 128:(f + 1) * 128],
                             pair(o1[kp])[:, :, :],
                             start=(kp == 0), stop=(kp == KP - 1), perf_mode=DR)
        xo = xf(f"x1_{f}")
        nc.vector.affine_then_add(xo[:], pp[:], xres_sb[f][:],
                                  msl(2)[:, f:f + 1], gb["sa"][:, f:f + 1])
        x1.append(xo)
        sq = sqt(f"sq_nc{f}")
        nc.vector.tensor_tensor(sq[:], xo[:], xo[:], ALU.mult)
        nc.tensor.matmul(nc_pss[:, 0:512], cst["ones128"][:], sq[:],
                         start=(f == 0), stop=(f == DT - 1))

    # =====================================================================
    # Stage 2: cross-attention sublayer
    # =====================================================================
    qk_norm(kca, M, load_sel("cksel"), "kca")
    xnc_p = [f8p(f"xncp{kp}") for kp in range(KP)]
    norm_mod(x1, T, seff["ca"], msl(sh_col["ca"]), "nc",
             lambda k: pair(xnc_p[k // 2])[:, k % 2, 0:T], pss=[nc_pss[:, 0:512]])
    q_ca = proj_dr("wcq", 0, xnc_p, T, cst["cqb"], 0, "qca", qt)
    qk_norm(q_ca, T, load_sel("cqsel"), "qca")
    wco_t = load_wgroup_dr("wco", 0, D, "wco")

    o2 = attention(q_ca, kca, vca, M, "a2")

    x2 = []
    n2_pss = psA("ssn_n2")
    for f in range(DT):
        pp = psB(f"po2_{f}")
        for kp in range(KP):
            nc.tensor.matmul(pp[:], pair(wco_t[kp])[:, :, f * 128:(f + 1) * 128],
                             pair(o2[kp])[:, :, :],
                             start=(kp == 0), stop=(kp == KP - 1), perf_mode=DR)
        xo = xf(f"x2_{f}")
        nc.vector.affine_then_add(xo[:], pp[:], x1[f][:],
                                  msl(5)[:, f:f + 1], gb["ca"][:, f:f + 1])
        x2.append(xo)
        sq = sqt(f"sq_n2{f}")
        nc.vector.tensor_tensor(sq[:], xo[:], xo[:], ALU.mult)
        nc.tensor.matmul(n2_pss[:, 0:512], cst["ones128"][:], sq[:],
                         start=(f == 0), stop=(f == DT - 1))

    # =====================================================================
    # Stage 3: SwiGLU FFN sublayer (e3m4 weights streamed, h in fp8e4)
    # =====================================================================
    xn2_p = [f8p(f"xn2p{kp}") for kp in range(KP)]
    norm_mod(x2, T, seff["ff"], msl(sh_col["ff"]), "n2",
             lambda k: pair(xn2_p[k // 2])[:, k % 2, 0:T], pss=[n2_pss[:, 0:512]])
    h_sb = []
    h2t = None
    for f0 in range(0, FHT, 8):
        nf = min(8, FHT - f0)
        w1t = load_wgroup_dr("w1", f0 * 128, nf * 128, f"w1_{f0}")
        w2t = load_wgroup_dr("w2", f0 * 128, nf * 128, f"w2_{f0}")
        for f in range(nf):
            fa = f0 + f
            pp1 = psB(f"ph1_{fa}")
            for kp in range(KP):
                nc.tensor.matmul(pp1[:],
                                 pair(w1t[kp])[:, :, f * 128:(f + 1) * 128],
                                 pair(xn2_p[kp])[:, :, 0:T],
                                 start=(kp == 0), stop=(kp == KP - 1),
                                 perf_mode=DR)
            h1tmp = sqt(f"h1t_{fa}")
            if sim_compat:
                h1a = sqt(f"h1a_{fa}")
                nc.scalar.activation(h1a[:], pp1[:], ACTF.Identity,
                                     bias=cst["b1f"][:, fa:fa + 1])
                nc.scalar.activation(h1tmp[:], pp1[:], ACTF.Sigmoid,
                                     bias=cst["b1f"][:, fa:fa + 1])
                nc.vector.tensor_tensor(h1tmp[:], h1tmp[:], h1a[:], ALU.mult)
            else:
                nc.scalar.activation(h1tmp[:], pp1[:], ACTF.Silu,
                                     bias=cst["b1f"][:, fa:fa + 1])
            pp2 = psB(f"ph2_{fa}")
            for kp in range(KP):
                nc.tensor.matmul(pp2[:],
                                 pair(w2t[kp])[:, :, f * 128:(f + 1) * 128],
                                 pair(xn2_p[kp])[:, :, 0:T],
                                 start=(kp == 0), stop=(kp == KP - 1),
                                 perf_mode=DR)
            if fa % 2 == 0:
                h2t = h2b(f"h2_{fa}")
                h2v = h2t[:, 0:512]
            else:
                h2v = h2t[:, 512:1024]
            nc.scalar.activation(h2v, pp2[:], ACTF.Identity,
                                 bias=cst["b2f"][:, fa:fa + 1])
            h1 = hsb(f"h_{fa}")
            nc.vector.tensor_tensor(h1[:], h1tmp[:], h2v, ALU.mult)
            h_sb.append(h1)

    # out = h @ w3: two passes of 4 feature tiles, 4 live psums each
    for fg in range(0, DT, 4):
        psf = [psB(f"pf{fg + f}") for f in range(4)]
        for k in range(FHT):
            w3t = wgt3(f"w3_{fg}_{k}", wid=512)
            nc.sync.dma_start(w3t[:, 0:512],
                              p["w3"][k * 128:(k + 1) * 128,
                                      fg * 128:(fg + 4) * 128])
            for f in range(4):
                nc.tensor.matmul(psf[f][:], w3t[:, f * 128:(f + 1) * 128],
                                 h_sb[k][:], start=(k == 0), stop=(k == FHT - 1))
        for f in range(4):
            xo = xf(f"xout{fg + f}")
            nc.vector.affine_then_add(
                xo[:], psf[f][:], x2[fg + f][:],
                g64["ff"][:, fg + f:fg + f + 1],
                gb["ff"][:, fg + f:fg + f + 1])
            nc.scalar.dma_start(p["out"][(fg + f) * 128:(fg + f + 1) * 128, :], xo[:])

    pg.release()
    ps.release()
    dram.release()


# ==========================================================================
# host side
# ==========================================================================

def _fm(vec):
    """[128*k] f32 vector -> feature-major [128, k] (col j = feature tile j)."""
    v = np.asarray(vec, np.float32)
    return np.ascontiguousarray(v.reshape(-1, 128).T)


def _bd16():
    bd = np.zeros((128, 128), np.float32)
    for t in range(8):
        for p_ in range(128):
            bd[p_, t * 16 + 2 * t + p_ // 64] = 1.0
    return bd.astype(BF16)


def _sel(weights64):
    """[16, 1024] selector: sel[i, t*128+p] = w[p%64] * (i == 2t + p//64)."""
    w = np.ones(64, np.float32) if weights64 is None else \
        np.asarray(weights64, np.float32)
    s = np.zeros((16, D), np.float32)
    for col in range(D):
        i = 2 * (col // 128) + (col % 128) // 64
        s[i, col] = w[col % 64]
    return s.astype(BF16)


def _sel2():
    s = np.zeros((1, 256), np.float32)
    s[0, 0:64] = 1.0
    s[0, 192:256] = 1.0
    return s.astype(BF16)


def _dr(w):
    """[K, F] -> k-pair interleaved [K//2, 2*F] fp8e4 for DoubleRow."""
    K, F = w.shape
    return np.ascontiguousarray(
        w.reshape(K // 256, 2, 128, F).transpose(0, 2, 1, 3).reshape(K // 2, 2 * F)
    ).astype(E4)


def _e3(w):
    return np.ascontiguousarray(np.asarray(w, np.float32) * WS).astype(E3)


def make_in_maps(inputs):
    f32 = lambda a: np.ascontiguousarray(np.asarray(a, np.float32))

    x = f32(inputs["x"]); src = f32(inputs["source_tokens"]); c = f32(inputs["c"])
    qkv_b = f32(inputs["sa_qkv_b"])
    o_w = f32(inputs["sa_o_w"]); o_b = f32(inputs["sa_o_b"])
    ckv_b = f32(inputs["ca_kv_b"])
    co_w = f32(inputs["ca_o_w"]); co_b = f32(inputs["ca_o_b"])
    w1 = f32(inputs["mlp_w1"]); b1 = f32(inputs["mlp_b1"])
    w2 = f32(inputs["mlp_w2"]); b2 = f32(inputs["mlp_b2"])
    w3 = f32(inputs["mlp_w3"]); b3 = f32(inputs["mlp_b3"])

    # pad SwiGLU hidden to 2816; zero pads keep silu(0)*0 == 0 exact
    w1p = np.zeros((D, MHP), np.float32); w1p[:, :MH] = w1
    w2p = np.zeros((D, MHP), np.float32); w2p[:, :MH] = w2
    w3p = np.zeros((MHP, D), np.float32); w3p[:MH, :] = w3
    b1p = np.zeros(MHP, np.float32); b1p[:MH] = b1
    b2p = np.zeros(MHP, np.float32); b2p[:MH] = b2

    # fold the V biases through the linear attention + output projection:
    # softmax(..) @ (v + vb) @ Wo = softmax(..) @ v @ Wo + vb @ Wo
    obf = qkv_b[2 * D:3 * D] @ o_w + o_b
    cobf = ckv_b[D:2 * D] @ co_w + co_b

    shared = dict(
        ada=_e3(f32(inputs["ada_w"])),
        wqkv=_dr(f32(inputs["sa_qkv_w"])),
        wo=_dr(o_w),
        wcq=_dr(f32(inputs["ca_q_w"])),
        wckv=_dr(f32(inputs["ca_kv_w"])),
        wco=_dr(co_w),
        w1=_dr(w1p), w2=_dr(w2p), w3=_e3(w3p),
        adab=_fm(f32(inputs["ada_b"])), n1w=_fm(f32(inputs["n1_w"])),
        ncw=_fm(f32(inputs["nc_w"])), n2w=_fm(f32(inputs["n2_w"])),
        qkvb=_fm(qkv_b), obf=_fm(obf), cqb=_fm(f32(inputs["ca_q_b"])),
        ckb=_fm(ckv_b[0:D]), cobf=_fm(cobf),
        b1f=_fm(b1p), b2f=_fm(b2p), b3f=_fm(b3),
        ones128=np.ones((128, 128), BF16),
        bd16=_bd16(),
        sel2=_sel2(),
        qsel=_sel(inputs["sa_qn_w"]), ksel=_sel(inputs["sa_kn_w"]),
        cqsel=_sel(inputs["ca_qn_w"]), cksel=_sel(inputs["ca_kn_w"]),
    )

    in_maps = []
    for cidx in range(NCORES):
        b, half = divmod(cidx, 2)
        xT = x[b].T  # [D, N]
        if half:
            xTp = np.concatenate([xT[:, T:], xT[:, :T]], axis=1)
        else:
            xTp = xT
        m = dict(shared)
        m["xt"] = np.ascontiguousarray(xTp).astype(BF16)
        m["xres"] = np.ascontiguousarray(xTp[:, :T])
        srcT = src[b].T.astype(np.float32)  # [D, M]
        m["srct"] = np.ascontiguousarray(
            srcT.reshape(4, 2, 128, M).transpose(0, 2, 1, 3).reshape(512, 2 * M)
        ).astype(E4)
        m["cvec"] = np.ascontiguousarray(c[b].reshape(D, 1))
        in_maps.append(m)
    return in_maps


def assemble(results):
    out = np.empty((B, N, D), np.float32)
    for cidx in range(NCORES):
        b, half = divmod(cidx, 2)
        out[b, half * T:(half + 1) * T, :] = results[cidx]["out"].T
    return out


_NC_CACHE = []


def kernel(**inputs):
    from concourse.bass_utils import run_bass_kernel_spmd
    if not _NC_CACHE:
        _NC_CACHE.append(build_graph())
    nc = _NC_CACHE[0]
    in_maps = make_in_maps(inputs)
    res = run_bass_kernel_spmd(nc, in_maps, core_ids=list(range(NCORES)))
    return assemble(res.results)


if __name__ == "__main__":
    nc = build_graph()
    print("graph built OK; instructions:",
          sum(len(bb.instructions) for bb in nc.main_func.blocks))


# revision 26
# speedup vs baseline: 1.0314x; 1.0314x over previous
"""Trainium2 Bass kernel for nn_ConditionalJiTBlock (DiT-style block with
AdaLN modulation, self-attention, cross-attention and SwiGLU FFN).

Sharding: 8 NeuronCores = 4 batch elements x 2 token-halves. Each core
computes its 512 query tokens end-to-end with zero collectives; the K/V
projections (which need all 1024 tokens of the batch element) are
replicated within each pair of cores. SPMD safety: the host permutes each
core's token axis so the core's local tokens are always columns 0..511 of
the on-chip tensors (attention is permutation-invariant over key tokens).

Layout: activations are feature-major on chip (features on partitions,
tokens on the free axis). Projection dtypes are mixed for speed at bounded
accuracy cost (validated against an fp64-path numpy sim):
  - qkv / cross-q / cross-kv weights + their activations: fp8e4m3 with
    DoubleRow matmuls (two contraction k-tiles per instruction).
  - ada / o-proj / cross-o / FFN weights: fp8e3m4 scaled x64 on host
    (weights are ~N(0, 0.02); the scale moves them out of e3m4's subnormal
    range), activations bf16/fp8; the 1/64 is folded into the PSUM
    evacuation affine. Attention q/k/scores/softmax/v stays bf16/f32.
Softmax: the two heads of a q/k tile are emitted back-to-back so their
score matmuls run concurrently in different PE row groups (base partition
0 / 64); exp is batched over 2-bank PSUM tiles; P@V chases the exp stream
one head-pair behind. The denominator comes from the interleaved
ones-column of V (PSUM row 64) and is broadcast across partitions with a
K=2 selector matmul.
"""

import numpy as np
import ml_dtypes

BF16 = ml_dtypes.bfloat16
E4 = ml_dtypes.float8_e4m3
E3 = ml_dtypes.float8_e3m4

B, N, M, D, H, HD = 4, 1024, 1024, 1024, 16, 64
MH = 2730
MHP = 2816          # MH padded to 22*128
EPS = 1e-6
NCORES = 8
T = 512             # local query tokens per core
DT = D // 128       # 8
KP = DT // 2        # 4 contraction k-pairs for DoubleRow
FHT = MHP // 128    # 22
NMOD = 9
ATT_SCALE = HD ** -0.5
WS = 64.0           # host scale applied to e3m4 weights


# ==========================================================================
# device graph
# ==========================================================================

def build_graph(sim_compat=False):
    import concourse.bacc as bacc
    import concourse.mybir as mybir
    import concourse.tile as tile

    F32 = mybir.dt.float32
    BT = mybir.dt.bfloat16
    F8 = mybir.dt.float8e4
    F8E3 = mybir.dt.float8e3

    nc = bacc.Bacc("TRN2", target_bir_lowering=False, debug=False,
                   num_devices=NCORES)

    def din(name, shape, dtype):
        return nc.dram_tensor(name, shape, dtype, kind="ExternalInput").ap()

    p = {}
    # activations
    p["xt"] = din("xt", [D, N], BT)          # x[b].T, local tokens first
    p["xres"] = din("xres", [D, T], F32)     # f32 residual columns (local)
    p["srct"] = din("srct", [512, 2 * M], F8)  # src^T, k-pair interleaved
    p["cvec"] = din("cvec", [D, 1], F32)     # c[b]
    # weights
    p["ada"] = din("ada", [D, NMOD * D], F8E3)      # x64
    p["wqkv"] = din("wqkv", [512, 2 * 3 * D], F8)   # k-pair interleaved
    p["wo"] = din("wo", [512, 2 * D], F8)           # k-pair interleaved
    p["wcq"] = din("wcq", [512, 2 * D], F8)
    p["wckv"] = din("wckv", [512, 2 * 2 * D], F8)
    p["wco"] = din("wco", [512, 2 * D], F8)         # k-pair interleaved
    p["w1"] = din("w1", [512, 2 * MHP], F8)         # k-pair interleaved
    p["w2"] = din("w2", [512, 2 * MHP], F8)         # k-pair interleaved
    p["w3"] = din("w3", [MHP, D], F8E3)             # x64
    # feature-major f32 vectors [128, k]  (column j = feature tile j)
    p["adab"] = din("adab", [128, NMOD * DT], F32)
    p["n1w"] = din("n1w", [128, DT], F32)
    p["ncw"] = din("ncw", [128, DT], F32)
    p["n2w"] = din("n2w", [128, DT], F32)
    p["qkvb"] = din("qkvb", [128, 3 * DT], F32)
    p["obf"] = din("obf", [128, DT], F32)    # sa_o_b + v_bias @ Wo (host fold)
    p["cqb"] = din("cqb", [128, DT], F32)
    p["ckb"] = din("ckb", [128, DT], F32)    # cross-k bias
    p["cobf"] = din("cobf", [128, DT], F32)  # ca_o_b + cross-v bias @ Wco
    p["b1f"] = din("b1f", [128, FHT], F32)
    p["b2f"] = din("b2f", [128, FHT], F32)
    p["b3f"] = din("b3f", [128, DT], F32)
    # constant selector matrices, bf16
    p["ones128"] = din("ones128", [128, 128], BT)
    p["bd16"] = din("bd16", [128, 128], BT)
    p["sel2"] = din("sel2", [1, 256], BT)
    p["qsel"] = din("qsel", [16, D], BT)
    p["ksel"] = din("ksel", [16, D], BT)
    p["cqsel"] = din("cqsel", [16, D], BT)
    p["cksel"] = din("cksel", [16, D], BT)

    p["out"] = nc.dram_tensor("out", [D, T], F32, kind="ExternalOutput").ap()

    with tile.TileContext(nc) as tc:
        _emit(nc, tc, p, mybir, sim_compat)
    nc.compile()
    return nc


def _emit(nc, tc, p, mybir, sim_compat=False):
    ALU = mybir.AluOpType
    ACTF = mybir.ActivationFunctionType
    DR = mybir.MatmulPerfMode.DoubleRow
    F32 = mybir.dt.float32
    BT = mybir.dt.bfloat16
    F8 = mybir.dt.float8e4
    F8E3 = mybir.dt.float8e3
    F8E5 = mybir.dt.float8e5

    pg = tc.alloc_tile_pool(name="pg", bufs=1)
    ps = tc.alloc_tile_pool(name="ps", bufs=1, space="PSUM")
    dram = tc.alloc_tile_pool(name="dram", bufs=1, space="DRAM")

    # shared-tag allocators; slots rotate by liveness within each tag
    def bigw(name):   # wide bf16 activations (xt / k / v tiles)
        return pg.tile([128, 1040], BT, tag="bigw", name=name, bufs=24)

    def xf(name):     # f32 [128, T] residual-stream tiles
        return pg.tile([128, T], F32, tag="xf", name=name, bufs=16)

    def qt(name):     # bf16 [128, T] q tiles
        return pg.tile([128, T], BT, tag="qt", name=name, bufs=8)

    def op8(name):    # attention-out pair tiles [128, 2*512] fp8e4
        return pg.tile([128, 2 * 512], F8, tag="op8", name=name, bufs=4)

    def f8p(name):    # fp8 k-pair activation tiles [128, 2*1024]
        return pg.tile([128, 2 * 1024], F8, tag="f8p", name=name, bufs=8)

    def wgt8(name):   # fp8e4 DoubleRow weight tiles [128, 2*1024]
        return pg.tile([128, 2 * 1024], F8, tag="wgt8", name=name, bufs=8)

    def wgt3(name, wid=1024):  # fp8e3 weight stream tiles
        return pg.tile([128, wid], F8E3, tag="wgt3", name=name, bufs=18)

    def pt2(name):    # exp(p) tiles [128, 1024] fp8e5 (kt-pair layout)
        return pg.tile([128, 1024], F8E5, tag="pt", name=name, bufs=6)

    def h2b(name):    # FFN h2 tiles [128, 1024] bf16
        return pg.tile([128, 1024], BT, tag="h2b", name=name, bufs=1)

    def vp8(name):    # paired V tiles [128, 2*1280] fp8e4 (kt-pair layout,
        # heads at stride 80 so DoubleRow weight offsets stay 16-aligned)
        return pg.tile([128, 2 * 1280], F8, tag="vp8", name=name, bufs=8)

    def hsb(name):    # FFN h tiles, fp8e4
        return pg.tile([128, T], F8, tag="h_sb", name=name, bufs=FHT)

    def sqt(name):    # bf16 [128, 512] scratch (squares, norm tmp, h1 tmp)
        return pg.tile([128, 512], BT, tag="sq", name=name, bufs=2)

    def scratch4k(name, rows=128, wid=1024):  # f32 scratch (rr/ssq)
        return pg.tile([rows, wid], F32, tag="s4k", name=name, bufs=1)

    def scrbf(name, rows=16, wid=1024):
        return pg.tile([rows, wid], BT, tag="sbf", name=name, bufs=1)

    def rbt(name):    # den broadcast tiles [128, T] bf16
        return pg.tile([128, T], BT, tag="rbt", name=name, bufs=1)

    def rdt(name):    # den strips [1, 2T] f32 (head A | head B)
        return pg.tile([1, 2 * T], F32, tag="rdt", name=name, bufs=1)

    def rdbt(name):   # den strips [1, 2T] bf16
        return pg.tile([1, 2 * T], BT, tag="rdbt", name=name, bufs=1)

    def selt(name):   # on-demand qk-norm selector tiles [16, D]
        return pg.tile([16, D], BT, tag="selt", name=name, bufs=2)

    def psA(name):    # 2-bank psum [128, 1024]
        return ps.tile([128, 1024], F32, tag="psA", name=name, bufs=2)

    def psB(name):    # 1-bank psum [128, 512]
        return ps.tile([128, 512], F32, tag="psB", name=name, bufs=4)

    def pair(t):
        """view [128, 2*W] tile as [128, 2, W]"""
        return t[:].rearrange("p (j f) -> p j f", j=2)

    # ---------------- constants ----------------
    cst = {}
    c_eps = pg.tile([128, 1], F32, tag="c_eps", name="c_eps")
    nc.any.memset(c_eps[:], EPS)
    for nm, np_, k in (("ones128", 128, 128), ("bd16", 128, 128),
                       ("sel2", 1, 256)):
        t = pg.tile([np_, k], BT, tag=nm, name=f"c_{nm}")
        nc.sync.dma_start(t[:], p[nm][:])
        cst[nm] = t
    for nm, k in (("adab", NMOD * DT), ("n1w", DT), ("ncw", DT), ("n2w", DT),
                  ("qkvb", 3 * DT), ("obf", DT), ("cqb", DT), ("ckb", DT),
                  ("cobf", DT), ("b1f", FHT), ("b2f", FHT), ("b3f", DT)):
        t = pg.tile([128, k], F32, tag=nm, name=f"c_{nm}")
        nc.sync.dma_start(t[:], p[nm][:])
        cst[nm] = t

    def load_sel(nm):
        t = selt(f"c_{nm}")
        nc.scalar.dma_start(t[:], p[nm][:])
        return t

    # =====================================================================
    # Stage 0: x/c loads first, then the ada stream (fp8e3).  RMS(x) stats
    # (mods-independent) are interleaved with the ada matmuls so the PE
    # stays busy while ada weight groups stream in.
    # =====================================================================
    xt_sb = []
    for k in range(DT):
        t = bigw(f"xt{k}")
        nc.sync.dma_start(t[:, 0:N], p["xt"][k * 128:(k + 1) * 128, :])
        xt_sb.append(t)
    cv = pg.tile([128, DT], F32, tag="cv", name="cv")
    nc.sync.dma_start(cv[:], p["cvec"][:].rearrange("(k p) o -> p (k o)", p=128))
    sc = pg.tile([128, DT], BT, tag="sc", name="sc")
    nc.scalar.activation(sc[:], cv[:], ACTF.Sigmoid)
    nc.vector.tensor_tensor(sc[:], sc[:], cv[:], ALU.mult)

    # n1 RMS sum-of-squares, interleaved unit-by-unit with the ada groups
    n1_pss = [psA("ssn_n1a"), psA("ssn_n1b")]

    def n1_units():
        for k in range(DT):
            for c in range(2):
                sq = sqt(f"sq_n1{k}_{c}")
                nc.vector.tensor_tensor(sq[:], xt_sb[k][:, c * 512:(c + 1) * 512],
                                        xt_sb[k][:, c * 512:(c + 1) * 512],
                                        ALU.mult)
                nc.tensor.matmul(n1_pss[c][:, 0:512], cst["ones128"][:], sq[:],
                                 start=(k == 0), stop=(k == DT - 1))
            yield

    n1_gen = n1_units()

    NCH_ADA = NMOD * D // 512  # 18
    scr = dram.tile([NMOD * D], F32, tag="scr", name="scr")
    scr_ch = scr[:].rearrange("(a b) -> a b", a=NCH_ADA)
    for grp in range(NMOD):  # 9 groups of 1024 ada columns
        gtiles = []
        for k in range(DT):
            at = wgt3(f"ada_g{grp}_{k}")
            nc.sync.dma_start(
                at[:], p["ada"][k * 128:(k + 1) * 128, grp * D:(grp + 1) * D])
            gtiles.append(at)
        for ch in range(2):
            gi = grp * 2 + ch
            pm = psB(f"pm{gi}")
            for k in range(DT):
                nc.tensor.matmul(pm[0:1, :], sc[:, k:k + 1],
                                 gtiles[k][:, ch * 512:(ch + 1) * 512],
                                 start=(k == 0), stop=(k == DT - 1))
            strip = pg.tile([1, 512], F32, tag="xf", name=f"mstrip{gi}",
                            bufs=16)
            nc.scalar.activation(strip[:], pm[0:1, :], ACTF.Identity,
                                 scale=1.0 / WS)
            nc.scalar.dma_start(scr_ch[gi:gi + 1, :], strip[:])
        next(n1_gen, None)

    for _ in n1_gen:
        pass

    seff = {}
    gb = {}
    g64 = {}
    _msl = []

    def emit_mods():
        """Reload mods feature-major, derive the per-sublayer vectors.
        Called after the n1 product pass so that pass overlaps the
        DRAM bounce."""
        mods = pg.tile([128, NMOD * DT], F32, tag="mods", name="mods")
        nc.scalar.dma_start(mods[:], scr[:].rearrange("(j p) -> p j", p=128))
        nc.vector.tensor_tensor(mods[:], mods[:], cst["adab"][:], ALU.add)

        def msl(i):  # mods columns of modulation param i
            return mods[:, i * DT:(i + 1) * DT]

        _msl.append(msl)
        for nm, i_scale, w in (("sa", 1, "n1w"), ("ca", 4, "ncw"),
                               ("ff", 7, "n2w")):
            s1 = pg.tile([128, DT], F32, tag=f"seff_{nm}", name=f"seff_{nm}")
            nc.vector.tensor_scalar(s1[:], msl(i_scale), 1.0, None, ALU.add)
            nc.vector.tensor_tensor(s1[:], s1[:], cst[w][:], ALU.mult)
            seff[nm] = s1
        for nm, i_gate, bias in (("sa", 2, "obf"), ("ca", 5, "cobf"),
                                 ("ff", 8, "b3f")):
            t = pg.tile([128, DT], F32, tag=f"gb_{nm}", name=f"gb_{nm}")
            nc.vector.tensor_tensor(t[:], msl(i_gate), cst[bias][:], ALU.mult)
            gb[nm] = t
        # gate columns pre-divided by WS (the e3m4 weight scale) for the
        # o-proj / cross-o / FFN-out PSUM evacuations
        for nm, i_gate in (("sa", 2), ("ca", 5), ("ff", 8)):
            t = pg.tile([128, DT], F32, tag=f"g64_{nm}", name=f"g64_{nm}")
            nc.vector.tensor_scalar(t[:], msl(i_gate), 1.0 / WS, None, ALU.mult)
            g64[nm] = t
        return msl

    sh_col = {"sa": 0, "ca": 3, "ff": 6}

    # =====================================================================
    # helpers
    # =====================================================================
    def load_wgroup_dr(w_name, cols0, cols, tagname):
        """fp8e4 DoubleRow weight tiles: KP tiles [128, 2, cols]."""
        tiles = []
        for kp in range(KP):
            t = wgt8(f"{tagname}_{kp}")
            nc.sync.dma_start(
                pair(t)[:, :, 0:cols],
                p[w_name][kp * 128:(kp + 1) * 128, :]
                .rearrange("p (j f) -> p j f", j=2)[:, :, cols0:cols0 + cols])
            tiles.append(t)
        return tiles

    def load_wgroup3(w_name, cols0, cols, tagname):
        """fp8e3 plain weight tiles, one per contraction k-tile."""
        tiles = []
        for k in range(DT):
            t = wgt3(f"{tagname}_{k}", wid=cols)
            nc.sync.dma_start(
                t[:, 0:cols], p[w_name][k * 128:(k + 1) * 128, cols0:cols0 + cols])
            tiles.append(t)
        return tiles

    def norm_mod(xtiles, Ttok, seff_t, sh_slice, name, writer, pss=None,
                 defer_mod=False):
        """RMS + AdaLN modulate of feature-major tiles.

        writer(k) -> destination AP [128, Ttok] for the k-tile.  The
        x*(1/rms) product is written first; the AdaLN affine is applied
        in place.  With defer_mod=True the affine pass is returned as a
        closure so the product pass can overlap work that does not yet
        have the modulation vectors.
        """
        NCH = Ttok // 512
        if pss is None:
            pp = psA(f"ssn_{name}")
            for k in range(DT):
                for c in range(NCH):
                    sq = sqt(f"sq_{name}{k}_{c}")
                    nc.vector.tensor_tensor(sq[:], xtiles[k][:, c * 512:(c + 1) * 512],
                                            xtiles[k][:, c * 512:(c + 1) * 512],
                                            ALU.mult)
                    nc.tensor.matmul(pp[:, c * 512:(c + 1) * 512],
                                     cst["ones128"][:], sq[:],
                                     start=(k == 0), stop=(k == DT - 1))
            pss = [pp[:, c * 512:(c + 1) * 512] for c in range(NCH)]
        rr = scratch4k(f"rr_{name}")
        for c in range(NCH):
            nc.scalar.activation(rr[:, c * 512:(c + 1) * 512], pss[c],
                                 ACTF.Sqrt, bias=c_eps[:], scale=1.0 / D)
        nc.vector.reciprocal_approx_fast(rr[:, 0:Ttok], rr[:, 0:Ttok])
        outs = []
        for k in range(DT):
            dst = writer(k)
            nc.vector.tensor_tensor(dst[:, 0:Ttok], xtiles[k][:, 0:Ttok],
                                    rr[:, 0:Ttok], ALU.mult)
            outs.append(dst)

        def modulate(seff_t, sh_slice):
            for k in range(DT):
                if k % 2 == 0:
                    nc.vector.tensor_scalar(outs[k][:, 0:Ttok], outs[k][:, 0:Ttok],
                                            seff_t[:, k:k + 1], sh_slice[:, k:k + 1],
                                            ALU.mult, ALU.add)
                else:
                    nc.scalar.activation(outs[k][:, 0:Ttok], outs[k][:, 0:Ttok],
                                         ACTF.Identity, bias=sh_slice[:, k:k + 1],
                                         scale=seff_t[:, k:k + 1])
            return outs

        if defer_mod:
            return modulate
        return modulate(seff_t, sh_slice)

    def qk_norm(qtiles, Ttok, sel_t, name, filler=None):
        """Per-head RMS norm in place; head-norm weight folded into sel."""
        NCH = Ttok // 512
        ssq = scratch4k(f"ssq_{name}", rows=16)
        for c in range(NCH):
            pq = psB(f"psq_{name}{c}")
            for t in range(DT):
                sq = sqt(f"qs_{name}{t}_{c}")
                nc.vector.tensor_tensor(sq[:], qtiles[t][:, c * 512:(c + 1) * 512],
                                        qtiles[t][:, c * 512:(c + 1) * 512],
                                        ALU.mult)
                nc.tensor.matmul(pq[0:16, :],
                                 cst["bd16"][:, t * 16:(t + 1) * 16], sq[:],
                                 start=(t == 0), stop=(t == DT - 1))
            nc.scalar.activation(ssq[:, c * 512:(c + 1) * 512], pq[0:16, :],
                                 ACTF.Sqrt, bias=c_eps[0:16, :], scale=1.0 / HD)
        nc.vector.reciprocal_approx_fast(ssq[:, 0:Ttok], ssq[:, 0:Ttok])
        rqb = scrbf(f"rqb_{name}")
        nc.vector.tensor_copy(rqb[:, 0:Ttok], ssq[:, 0:Ttok])
        for t in range(DT):
            for c in range(NCH):
                pb = psB(f"qb_{name}{t}_{c}")
                nc.tensor.matmul(pb[:], sel_t[:, t * 128:(t + 1) * 128],
                                 rqb[:, c * 512:(c + 1) * 512],
                                 start=True, stop=True)
                nc.vector.tensor_tensor(qtiles[t][:, c * 512:(c + 1) * 512],
                                        qtiles[t][:, c * 512:(c + 1) * 512],
                                        pb[:], ALU.mult)
            if filler is not None:
                filler()

    def attention(q_sb, k_sb, v_sb, Tk, name, filler=None):
        """softmax(q k^T / 8) v with a (t, kp)-unit software pipeline."""
        KTk = Tk // 128
        NKP = KTk // 2
        o_pair = [op8(f"o_{name}{m}") for m in range(DT // 2)]

        def o_half(t):
            return o_pair[t // 2][:, (t % 2) * 512:(t % 2) * 512 + 512]
        pts = {}
        po_cur = {}
        rd_cur = {}

        def emit_S(t, kp):
            spA = psA(f"sA_{name}{t}_{kp}")
            spB = psA(f"sB_{name}{t}_{kp}")
            for j in (0, 1):
                kt = 2 * kp + j
                nc.tensor.matmul(spA[:, j * 512:(j + 1) * 512],
                                 k_sb[t][0:64, kt * 128:(kt + 1) * 128],
                                 q_sb[t][0:64, 0:T], start=True, stop=True)
                nc.tensor.matmul(spB[:, j * 512:(j + 1) * 512],
                                 k_sb[t][64:128, kt * 128:(kt + 1) * 128],
                                 q_sb[t][64:128, 0:T], start=True, stop=True)
            pA = pt2(f"ptA_{name}{t}_{kp}")
            pB = pt2(f"ptB_{name}{t}_{kp}")
            nc.scalar.activation(pA[:], spA[:], ACTF.Exp, scale=ATT_SCALE)
            nc.scalar.activation(pB[:], spB[:], ACTF.Exp, scale=ATT_SCALE)
            pts[(t, kp)] = (pA, pB)

        def emit_PV(t, kp):
            if kp == 0:
                po_cur[t] = (psB(f"poA_{name}{t}"), psB(f"poB_{name}{t}"))
            poA, poB = po_cur[t]
            pA, pB = pts.pop((t, kp))
            for po, pt_, hp in ((poA, pA, 0), (poB, pB, 1)):
                h16 = 2 * t + hp
                nc.tensor.matmul(po[0:65, :],
                                 pair(v_sb[kp])[:, :, h16 * 80:h16 * 80 + 65],
                                 pt_[:].rearrange("p (j f) -> p j f", j=2),
                                 start=(kp == 0), stop=(kp == NKP - 1),
                                 perf_mode=DR)

        def emit_EPI(t):
            # NOTE: custom DVE ops (reciprocal_approx_fast) ignore the
            # operand base partition on HW — copy PSUM row 64 to a
            # partition-0 strip first, then recip in place.
            poA, poB = po_cur.pop(t)
            rd = rdt(f"rd_{name}{t}")
            nc.vector.tensor_copy(rd[:, 0:T], poA[64:65, 0:T])
            nc.vector.tensor_copy(rd[:, T:2 * T], poB[64:65, 0:T])
            nc.vector.reciprocal_approx_fast(rd[:], rd[:])
            rdb = rdbt(f"rdb_{name}{t}")
            nc.vector.tensor_copy(rdb[:], rd[:])
            bc = psB(f"bc_{name}{t}")
            nc.tensor.matmul(bc[:], cst["sel2"][:, 0:128], rdb[:, 0:T],
                             start=True, stop=False)
            nc.tensor.matmul(bc[:], cst["sel2"][:, 128:256], rdb[:, T:2 * T],
                             start=False, stop=True)
            rb = rbt(f"rb_{name}{t}")
            nc.scalar.activation(rb[:], bc[:], ACTF.Identity)
            oh = o_half(t)
            nc.vector.tensor_tensor(oh[0:64, :], poA[0:64, :],
                                    rb[0:64, :], ALU.mult)
            nc.vector.tensor_tensor(oh[64:128, :], poB[0:64, :],
                                    rb[64:128, :], ALU.mult)

        units = [(t, kp) for t in range(DT) for kp in range(NKP)]
        for i in range(len(units) + 2):
            if i < len(units):
                emit_S(*units[i])
            if i >= 2:
                t, kp = units[i - 2]
                emit_PV(t, kp)
                if kp == NKP - 1:
                    emit_EPI(t)
                if filler is not None:
                    filler()
        return o_pair

    def proj_dr(w_name, wcols0, xn_p, Tt, bias, bias0, name, alloc, n_f=DT):
        """Feature-major fp8 DoubleRow projection over n_f output tiles."""
        outs = []
        NCH = Tt // 512
        for f0 in range(0, n_f, 8):
            nf = min(8, n_f - f0)
            wt = load_wgroup_dr(w_name, wcols0 + f0 * 128, nf * 128,
                                f"{name}_w{f0}")
            for f in range(nf):
                o = alloc(f"{name}_o{f0 + f}")
                outs.append(o)
                for c in range(NCH):
                    pp = psB(f"p_{name}{f0 + f}_{c}")
                    for kp in range(KP):
                        nc.tensor.matmul(
                            pp[:],
                            pair(wt[kp])[:, :, f * 128:(f + 1) * 128],
                            pair(xn_p[kp])[:, :, c * 512:(c + 1) * 512],
                            start=(kp == 0), stop=(kp == KP - 1),
                            perf_mode=DR)
                    nc.scalar.activation(
                        o[:, c * 512:(c + 1) * 512], pp[:], ACTF.Identity,
                        bias=bias[:, bias0 + f0 + f:bias0 + f0 + f + 1])
        return outs

    def proj_tok_dr(w_name, wcols0, xn_p, Tt, name):
        """Token-major fp8 DoubleRow V projection into kt-paired tiles
        with an interleaved ones column per head (position 64 of each
        80-wide head block)."""
        ntt = Tt // 128
        wt = load_wgroup_dr(w_name, wcols0, D, f"{name}_w")
        outs = []
        for m in range(ntt // 2):
            o = vp8(f"{name}_v{m}")
            nc.any.memset(o[:], 1.0)
            outs.append(o)
        for tt in range(ntt):
            for c in range(2):
                pp = psB(f"pv_{name}{tt}_{c}")
                for kp in range(KP):
                    nc.tensor.matmul(
                        pp[:],
                        pair(xn_p[kp])[:, :, tt * 128:(tt + 1) * 128],
                        pair(wt[kp])[:, :, c * 512:(c + 1) * 512],
                        start=(kp == 0), stop=(kp == KP - 1),
                        perf_mode=DR)
                half = outs[tt // 2][:, (tt % 2) * 1280:(tt % 2) * 1280 + 1280]
                dst = half[:, c * 8 * 80:(c * 8 + 8) * 80].rearrange(
                    "p (g e) -> p g e", g=8)[:, :, 0:64]
                src = pp[:].rearrange("p (g e) -> p g e", g=8)
                nc.vector.tensor_copy(dst, src)
        return outs

    # =====================================================================
    # Stage 1: self-attention sublayer
    # =====================================================================
    # normalized x in fp8 k-pair layout for the DoubleRow projections;
    # the x*(1/rms) pass runs while the mods DRAM bounce is in flight
    xn_p = [f8p(f"xnp{kp}") for kp in range(KP)]
    n1_modulate = norm_mod(xt_sb, N, None, None, "n1",
                           lambda k: pair(xn_p[k // 2])[:, k % 2, :],
                           pss=[n1_pss[c][:, 0:512] for c in range(2)],
                           defer_mod=True)
    msl = emit_mods()
    n1_modulate(seff["sa"], msl(sh_col["sa"]))

    q_sa = proj_dr("wqkv", 0, xn_p, T, cst["qkvb"], 0, "qsa", qt)

    xres_sb = []
    for k in range(DT):
        t = xf(f"xres{k}")
        nc.scalar.dma_start(t[:], p["xres"][k * 128:(k + 1) * 128, :])
        xres_sb.append(t)
    k_sa = proj_dr("wqkv", D, xn_p, N, cst["qkvb"], DT, "ksa", bigw)
    v_sa = proj_tok_dr("wqkv", 2 * D, xn_p, N, "vsa")

    # source tokens + cross-KV as filler units (qk-norm gaps + attention-1)
    srct_p = []
    for kp in range(KP):
        t = f8p(f"srctp{kp}")
        nc.scalar.dma_start(t[:, 0:2 * M], p["srct"][kp * 128:(kp + 1) * 128, :])
        srct_p.append(t)

    kca = [bigw(f"kca_o{f}") for f in range(DT)]
    vca = []
    for m in range(M // 256):
        o = vp8(f"vca_v{m}")
        nc.any.memset(o[:], 1.0)
        vca.append(o)

    def ckv_units():
        wt = load_wgroup_dr("wckv", 0, D, "kca_w")
        for f in range(DT):
            for c in range(2):
                pp = psB(f"p_kca{f}_{c}")
                for kp in range(KP):
                    nc.tensor.matmul(
                        pp[:], pair(wt[kp])[:, :, f * 128:(f + 1) * 128],
                        pair(srct_p[kp])[:, :, c * 512:(c + 1) * 512],
                        start=(kp == 0), stop=(kp == KP - 1), perf_mode=DR)
                nc.scalar.activation(kca[f][:, c * 512:(c + 1) * 512], pp[:],
                                     ACTF.Identity, bias=cst["ckb"][:, f:f + 1])
                yield
        wtv = load_wgroup_dr("wckv", D, D, "vca_w")
        for tt in range(M // 128):
            for c in range(2):
                pp = psB(f"pv_vca{tt}_{c}")
                for kp in range(KP):
                    nc.tensor.matmul(
                        pp[:], pair(srct_p[kp])[:, :, tt * 128:(tt + 1) * 128],
                        pair(wtv[kp])[:, :, c * 512:(c + 1) * 512],
                        start=(kp == 0), stop=(kp == KP - 1), perf_mode=DR)
                half = vca[tt // 2][:, (tt % 2) * 1280:(tt % 2) * 1280 + 1280]
                dst = half[:, c * 8 * 80:(c * 8 + 8) * 80].rearrange(
                    "p (g e) -> p g e", g=8)[:, :, 0:64]
                nc.vector.tensor_copy(dst, pp[:].rearrange("p (g e) -> p g e", g=8))
                yield

    ckv_gen = ckv_units()

    def ckv_filler():
        next(ckv_gen, None)

    qk_norm(q_sa, T, load_sel("qsel"), "qsa", filler=ckv_filler)
    qk_norm(k_sa, N, load_sel("ksel"), "ksa", filler=ckv_filler)

    wo_t = load_wgroup_dr("wo", 0, D, "wo")
    o1 = attention(q_sa, k_sa, v_sa, N, "a1", filler=ckv_filler)
    for _ in ckv_gen:
        pass

    x1 = []
    nc_pss = psA("ssn_nc")
    for f in range(DT):
        pp = psB(f"po1_{f}")
        for kp in range(KP):
            nc.tensor.matmul(pp[:], pair(wo_t[kp])[:, :, f * 128:(f + 1) * 128],
                             pair(o1[kp])[:, :, :],
                             start=(kp == 0), stop=(kp == KP - 1), perf_mode=DR)
        xo = xf(f"x1_{f}")
        nc.vector.affine_then_add(xo[:], pp[:], xres_sb[f][:],
                                  msl(2)[:, f:f + 1], gb["sa"][:, f:f + 1])
        x1.append(xo)
        sq = sqt(f"sq_nc{f}")
        nc.vector.tensor_tensor(sq[:], xo[:], xo[:], ALU.mult)
        nc.tensor.matmul(nc_pss[:, 0:512], cst["ones128"][:], sq[:],
                         start=(f == 0), stop=(f == DT - 1))

    # =====================================================================
    # Stage 2: cross-attention sublayer
    # =====================================================================
    qk_norm(kca, M, load_sel("cksel"), "kca")
    xnc_p = [f8p(f"xncp{kp}") for kp in range(KP)]
    norm_mod(x1, T, seff["ca"], msl(sh_col["ca"]), "nc",
             lambda k: pair(xnc_p[k // 2])[:, k % 2, 0:T], pss=[nc_pss[:, 0:512]])
    q_ca = proj_dr("wcq", 0, xnc_p, T, cst["cqb"], 0, "qca", qt)
    qk_norm(q_ca, T, load_sel("cqsel"), "qca")
    wco_t = load_wgroup_dr("wco", 0, D, "wco")

    o2 = attention(q_ca, kca, vca, M, "a2")

    x2 = []
    n2_pss = psA("ssn_n2")
    for f in range(DT):
        pp = psB(f"po2_{f}")
        for kp in range(KP):
            nc.tensor.matmul(pp[:], pair(wco_t[kp])[:, :, f * 128:(f + 1) * 128],
                             pair(o2[kp])[:, :, :],
                             start=(kp == 0), stop=(kp == KP - 1), perf_mode=DR)
        xo = xf(f"x2_{f}")
        nc.vector.affine_then_add(xo[:], pp[:], x1[f][:],
                                  msl(5)[:, f:f + 1], gb["ca"][:, f:f + 1])
        x2.append(xo)
        sq = sqt(f"sq_n2{f}")
        nc.vector.tensor_tensor(sq[:], xo[:], xo[:], ALU.mult)
        nc.tensor.matmul(n2_pss[:, 0:512], cst["ones128"][:], sq[:],
                         start=(f == 0), stop=(f == DT - 1))

    # =====================================================================
    # Stage 3: SwiGLU FFN sublayer (e3m4 weights streamed, h in fp8e4)
    # =====================================================================
    xn2_p = [f8p(f"xn2p{kp}") for kp in range(KP)]
    norm_mod(x2, T, seff["ff"], msl(sh_col["ff"]), "n2",
             lambda k: pair(xn2_p[k // 2])[:, k % 2, 0:T], pss=[n2_pss[:, 0:512]])
    h_sb = []
    h2t = None
    for f0 in range(0, FHT, 8):
        nf = min(8, FHT - f0)
        w1t = load_wgroup_dr("w1", f0 * 128, nf * 128, f"w1_{f0}")
        w2t = load_wgroup_dr("w2", f0 * 128, nf * 128, f"w2_{f0}")
        for f in range(nf):
            fa = f0 + f
            pp1 = psB(f"ph1_{fa}")
            for kp in range(KP):
                nc.tensor.matmul(pp1[:],
                                 pair(w1t[kp])[:, :, f * 128:(f + 1) * 128],
                                 pair(xn2_p[kp])[:, :, 0:T],
                                 start=(kp == 0), stop=(kp == KP - 1),
                                 perf_mode=DR)
            h1tmp = sqt(f"h1t_{fa}")
            if sim_compat:
                h1a = sqt(f"h1a_{fa}")
                nc.scalar.activation(h1a[:], pp1[:], ACTF.Identity,
                                     bias=cst["b1f"][:, fa:fa + 1])
                nc.scalar.activation(h1tmp[:], pp1[:], ACTF.Sigmoid,
                                     bias=cst["b1f"][:, fa:fa + 1])
                nc.vector.tensor_tensor(h1tmp[:], h1tmp[:], h1a[:], ALU.mult)
            else:
                nc.scalar.activation(h1tmp[:], pp1[:], ACTF.Silu,
                                     bias=cst["b1f"][:, fa:fa + 1])
            pp2 = psB(f"ph2_{fa}")
            for kp in range(KP):
                nc.tensor.matmul(pp2[:],
                                 pair(w2t[kp])[:, :, f * 128:(f + 1) * 128],
                                 pair(xn2_p[kp])[:, :, 0:T],
                                 start=(kp == 0), stop=(kp == KP - 1),
                                 perf_mode=DR)
            if fa % 2 == 0:
                h2t = h2b(f"h2_{fa}")
                h2v = h2t[:, 0:512]
            else:
                h2v = h2t[:, 512:1024]
            nc.scalar.activation(h2v, pp2[:], ACTF.Identity,
                                 bias=cst["b2f"][:, fa:fa + 1])
            h1 = hsb(f"h_{fa}")
            nc.vector.tensor_tensor(h1[:], h1tmp[:], h2v, ALU.mult)
            h_sb.append(h1)

    # out = h @ w3: two passes of 4 feature tiles, 4 live psums each
    for fg in range(0, DT, 4):
        psf = [psB(f"pf{fg + f}") for f in range(4)]
        for k in range(FHT):
            w3t = wgt3(f"w3_{fg}_{k}", wid=512)
            nc.sync.dma_start(w3t[:, 0:512],
                              p["w3"][k * 128:(k + 1) * 128,
                                      fg * 128:(fg + 4) * 128])
            for f in range(4):
                nc.tensor.matmul(psf[f][:], w3t[:, f * 128:(f + 1) * 128],
                                 h_sb[k][:], start=(k == 0), stop=(k == FHT - 1))
        for f in range(4):
            xo = xf(f"xout{fg + f}")
            nc.vector.affine_then_add(
                xo[:], psf[f][:], x2[fg + f][:],
                g64["ff"][:, fg + f:fg + f + 1],
                gb["ff"][:, fg + f:fg + f + 1])
            nc.scalar.dma_start(p["out"][(fg + f) * 128:(fg + f + 1) * 128, :], xo[:])

    pg.release()
    ps.release()
    dram.release()


# ==========================================================================
# host side
# ==========================================================================

def _fm(vec):
    """[128*k] f32 vector -> feature-major [128, k] (col j = feature tile j)."""
    v = np.asarray(vec, np.float32)
    return np.ascontiguousarray(v.reshape(-1, 128).T)


def _bd16():
    bd = np.zeros((128, 128), np.float32)
    for t in range(8):
        for p_ in range(128):
            bd[p_, t * 16 + 2 * t + p_ // 64] = 1.0
    return bd.astype(BF16)


def _sel(weights64):
    """[16, 1024] selector: sel[i, t*128+p] = w[p%64] * (i == 2t + p//64)."""
    w = np.ones(64, np.float32) if weights64 is None else \
        np.asarray(weights64, np.float32)
    s = np.zeros((16, D), np.float32)
    for col in range(D):
        i = 2 * (col // 128) + (col % 128) // 64
        s[i, col] = w[col % 64]
    return s.astype(BF16)


def _sel2():
    s = np.zeros((1, 256), np.float32)
    s[0, 0:64] = 1.0
    s[0, 192:256] = 1.0
    return s.astype(BF16)


def _dr(w):
    """[K, F] -> k-pair interleaved [K//2, 2*F] fp8e4 for DoubleRow."""
    K, F = w.shape
    return np.ascontiguousarray(
        w.reshape(K // 256, 2, 128, F).transpose(0, 2, 1, 3).reshape(K // 2, 2 * F)
    ).astype(E4)


def _e3(w):
    return np.ascontiguousarray(np.asarray(w, np.float32) * WS).astype(E3)


def make_in_maps(inputs):
    f32 = lambda a: np.ascontiguousarray(np.asarray(a, np.float32))

    x = f32(inputs["x"]); src = f32(inputs["source_tokens"]); c = f32(inputs["c"])
    qkv_b = f32(inputs["sa_qkv_b"])
    o_w = f32(inputs["sa_o_w"]); o_b = f32(inputs["sa_o_b"])
    ckv_b = f32(inputs["ca_kv_b"])
    co_w = f32(inputs["ca_o_w"]); co_b = f32(inputs["ca_o_b"])
    w1 = f32(inputs["mlp_w1"]); b1 = f32(inputs["mlp_b1"])
    w2 = f32(inputs["mlp_w2"]); b2 = f32(inputs["mlp_b2"])
    w3 = f32(inputs["mlp_w3"]); b3 = f32(inputs["mlp_b3"])

    # pad SwiGLU hidden to 2816; zero pads keep silu(0)*0 == 0 exact
    w1p = np.zeros((D, MHP), np.float32); w1p[:, :MH] = w1
    w2p = np.zeros((D, MHP), np.float32); w2p[:, :MH] = w2
    w3p = np.zeros((MHP, D), np.float32); w3p[:MH, :] = w3
    b1p = np.zeros(MHP, np.float32); b1p[:MH] = b1
    b2p = np.zeros(MHP, np.float32); b2p[:MH] = b2

    # fold the V biases through the linear attention + output projection:
    # softmax(..) @ (v + vb) @ Wo = softmax(..) @ v @ Wo + vb @ Wo
    obf = qkv_b[2 * D:3 * D] @ o_w + o_b
    cobf = ckv_b[D:2 * D] @ co_w + co_b

    shared = dict(
        ada=_e3(f32(inputs["ada_w"])),
        wqkv=_dr(f32(inputs["sa_qkv_w"])),
        wo=_dr(o_w),
        wcq=_dr(f32(inputs["ca_q_w"])),
        wckv=_dr(f32(inputs["ca_kv_w"])),
        wco=_dr(co_w),
        w1=_dr(w1p), w2=_dr(w2p), w3=_e3(w3p),
        adab=_fm(f32(inputs["ada_b"])), n1w=_fm(f32(inputs["n1_w"])),
        ncw=_fm(f32(inputs["nc_w"])), n2w=_fm(f32(inputs["n2_w"])),
        qkvb=_fm(qkv_b), obf=_fm(obf), cqb=_fm(f32(inputs["ca_q_b"])),
        ckb=_fm(ckv_b[0:D]), cobf=_fm(cobf),
        b1f=_fm(b1p), b2f=_fm(b2p), b3f=_fm(b3),
        ones128=np.ones((128, 128), BF16),
        bd16=_bd16(),
        sel2=_sel2(),
        qsel=_sel(inputs["sa_qn_w"]), ksel=_sel(inputs["sa_kn_w"]),
        cqsel=_sel(inputs["ca_qn_w"]), cksel=_sel(inputs["ca_kn_w"]),
    )

    in_maps = []
    for cidx in range(NCORES):
        b, half = divmod(cidx, 2)
        xT = x[b].T  # [D, N]
        if half:
            xTp = np.concatenate([xT[:, T:], xT[:, :T]], axis=1)
        else:
            xTp = xT
        m = dict(shared)
        m["xt"] = np.ascontiguousarray(xTp).astype(BF16)
        m["xres"] = np.ascontiguousarray(xTp[:, :T])
        srcT = src[b].T.astype(np.float32)  # [D, M]
        m["srct"] = np.ascontiguousarray(
            srcT.reshape(4, 2, 128, M).transpose(0, 2, 1, 3).reshape(512, 2 * M)
        ).astype(E4)
        m["cvec"] = np.ascontiguousarray(c[b].reshape(D, 1))
        in_maps.append(m)
    return in_maps


def assemble(results):
    out = np.empty((B, N, D), np.float32)
    for cidx in range(NCORES):
        b, half = divmod(cidx, 2)
        out[b, half * T:(half + 1) * T, :] = results[cidx]["out"].T
    return out


_NC_CACHE = []


def kernel(**inputs):
    from concourse.bass_utils import run_bass_kernel_spmd
    if not _NC_CACHE:
        _NC_CACHE.append(build_graph())
    nc = _NC_CACHE[0]
    in_maps = make_in_maps(inputs)
    res = run_bass_kernel_spmd(nc, in_maps, core_ids=list(range(NCORES)))
    return assemble(res.results)


if __name__ == "__main__":
    nc = build_graph()
    print("graph built OK; instructions:",
          sum(len(bb.instructions) for bb in nc.main_func.blocks))
